# revision 1
# baseline (speedup 1.0000x reference)
"""Trainium2 Bass/Tile kernel for the gnn_message_passing problem.

Math (per batch element b, with x = ftr[b] viewed as [C, HW], X = x^T):
    avg[c] = mean_n x[c,n];  mx[c] = max_n x[c,n]
    cw     = sigmoid(relu(Wa @ avg) + relu(Wm @ mx))              [M]
    k      = relu(Wk @ x + bk)                                    [M, HW]
    kq     = cw[:,None] * k
    S      = sigmoid(kq^T k)   (symmetric!)                       [HW, HW]
    d      = (S @ 1)^(-1/2)                                       [HW]
    out^T  = X(I+g) + dia(d) (k^T agc)                            [HW, C]
    agc    = -cw (.) B,  B = kdT^T uT,  kdT = dia(d) k^T,  uT = X g

S is symmetric: only upper-triangle strips are computed.  Row-sums via ACT
sigmoid+accumulate; lower-triangle contributions are per-strip column sums
(PE ones-column matmuls) into a persistent PSUM accumulator G, reduced at the
end by ones-matvecs.  The A/AG chain of the reference is folded into
B = kdT^T (x^T g), so x never needs a full transpose.

Output is written transposed ([HW, C] DRAM) so the tail stays in matmul
layouts; the host transposes back (free).

PSUM (8 banks): score arena [P,1536] (3 banks, two 768 halves, manual
rotation with subtile deps) + G [P,2304] (5 banks).  The tail tile aliases
G's banks via tag rotation.  Head matmuls share the arena halves.

Cross-rep pipelining: pools are hoisted; per-rep tiles use bufs=2 tags so
rep k+1's head (DMA, pools, k, cw, kq, kT, uT) overlaps rep k's score/tail.

Sharding: data-parallel over batch B=8 across 8 cores (1 image per core),
weights replicated. No collectives.
"""

import numpy as np
from contextlib import ExitStack

import concourse.bass as bass
import concourse.mybir as mybir
import concourse.tile as tile
from concourse import bacc
from concourse.bass_utils import run_bass_kernel_spmd
from concourse.masks import make_identity

F32 = mybir.dt.float32
F32R = mybir.dt.float32r  # fp32 bits, reduced-precision 4x-faster PE mode
BF16 = mybir.dt.bfloat16
AF = mybir.ActivationFunctionType
AX = mybir.AxisListType
OP = mybir.AluOpType

B, C, H, W = 8, 256, 48, 48
HW = H * W            # 2304
M = 128               # C // 2
P = 128               # partitions
CT = C // P           # 2 c-tiles
NT = HW // P          # 18 n-tiles
N_CORES = 8
BANK = 512            # fp32 elements per PSUM bank
SLOT = 768            # arena half size (1.5 banks)


def _chunks(total, step, start=0):
    out = []
    off = start
    while off < total:
        sz = min(step, total - off)
        out.append((off, sz))
        off += sz
    return out


def _bank_chunks(start, end):
    """[start, end) split at PSUM bank boundaries."""
    out = []
    off = start
    while off < end:
        nxt = min(end, (off // BANK + 1) * BANK)
        out.append((off, nxt - off))
        off = nxt
    return out


def _slot_pieces(h, sz):
    """Bank-aligned matmul piece list (off, len) within a SLOT at half h."""
    base = [(0, 512), (512, 256)] if h == 0 else [(0, 256), (256, 512)]
    out = []
    for o, ln in base:
        if o >= sz:
            break
        out.append((o, min(ln, sz - o)))
    return out


def build_program(reps=1, triangle=None):
    nc = bacc.Bacc("TRN2", target_bir_lowering=False, debug=False)

    ftr = nc.declare_dram_parameter("ftr", [C, HW], F32, isOutput=False)
    convw = nc.declare_dram_parameter("convw", [M, C], F32, isOutput=False)
    convb = nc.declare_dram_parameter("convb", [M, 1], F32, isOutput=False)
    avgw = nc.declare_dram_parameter("avgw", [M, C], F32, isOutput=False)
    maxw = nc.declare_dram_parameter("maxw", [M, C], F32, isOutput=False)
    gcnw = nc.declare_dram_parameter("gcnw", [C, C], F32, isOutput=False)
    out = nc.declare_dram_parameter("out", [P, NT * C], F32, isOutput=True)

    with tile.TileContext(nc) as tc, ExitStack() as ctx:
        sb = ctx.enter_context(tc.tile_pool(name="sb", bufs=1))
        scr = ctx.enter_context(tc.tile_pool(name="scr", bufs=3))
        apool = ctx.enter_context(tc.tile_pool(name="apool", bufs=3, space="PSUM"))
        gpool = ctx.enter_context(tc.tile_pool(name="gpool", bufs=1, space="PSUM"))

        # constants (built once, never rewritten)
        ident = sb.tile([P, P], F32, tag="ident")
        q_sb = sb.tile([P, 2 * P], BF16, tag="q")
        ones_sb = sb.tile([P, 1], F32, tag="ones")
        identr = sb.tile([P, P], F32R, tag="identr")
        make_identity(nc, ident)
        nc.vector.tensor_copy(identr, ident)
        nc.vector.memset(q_sb, 0.0)
        nc.vector.memset(q_sb[:, P - 1:P], 1.0)
        nc.vector.memset(ones_sb, 1.0)
        consts = (ident, identr, q_sb, ones_sb)

        drams = (ftr, convw, convb, avgw, maxw, gcnw, out)
        pools = (sb, scr, apool, gpool)

        cur, ops0 = _emit_head(tc, pools, consts, drams)
        for op in ops0:
            op()
        for r in range(reps):
            if r + 1 < reps:
                nxt, nops = _emit_head(tc, pools, consts, drams)
            else:
                nxt, nops = None, []
            _emit_score(tc, pools, consts, cur, nops)
            for op in nops:
                op()
            nops.clear()
            _emit_tail(tc, pools, consts, cur, drams)
            cur = nxt
    nc.compile()
    return nc


class _Rep:
    pass


def _emit_head(tc, pools, consts, drams):
    """Allocate a rep's tiles, emit its DMAs, and return closures for the
    head compute (interleaved into the PREVIOUS rep's score loop)."""
    nc = tc.nc
    sb, scr, apool, gpool = pools
    ident, identr, q_sb, ones_sb = consts
    ftr, convw, convb, avgw, maxw, gcnw, out = drams

    def t2(shape, dtype, tag):
        return sb.tile(shape, dtype, tag=tag, bufs=2, name=tag)

    def t1(shape, dtype, tag):
        return sb.tile(shape, dtype, tag=tag, bufs=1, name=tag)

    r = _Rep()
    r.xr = t2([P, CT, HW], F32R, "xr")
    r.k = t2([P, HW], F32R, "k")
    r.kq = t2([P, HW], F32R, "kq")
    r.kT = t2([P, NT, M], BF16, "kT")
    r.uT = t2([P, NT, C], BF16, "uT")
    r.gxT = t2([P, NT, C], BF16, "gxT")
    r.dcol = t2([P, 20], F32, "dcol")
    r.outT = t1([P, NT, C], F32, "outT")
    r.G = t1([P, HW], F32, "G")
    r.convw = t2([P, C], F32, "convw")
    r.convwT = t2([P, CT, M], F32R, "convwT")
    r.convb = t2([P, 1], F32, "convb")
    r.avgw = t2([P, C], F32, "avgw")
    r.avgwT = t2([P, CT, M], F32, "avgwT")
    r.maxw = t2([P, C], F32, "maxw")
    r.maxwT = t2([P, CT, M], F32, "maxwT")
    r.g = t1([P, CT, C], F32, "g")
    r.gr = t2([P, CT, C], F32R, "gr")
    r.gplus = t2([P, CT, C], F32R, "gplus")
    r.xfold = t1([P, CT, HW // 2], F32, "xfold")
    r.mfold = t1([P, CT, HW // 2], F32, "mfold")
    r.avg = t2([P, CT, 1], F32, "avg")
    r.mx = t2([P, CT, 1], F32, "mx")
    r.ra = t2([P, 1], F32, "ra")
    r.rm = t2([P, 1], F32, "rm")
    r.cwin = t2([P, 1], F32, "cwin")
    r.cw = t2([P, 1], F32, "cw")
    r.ncw = t2([P, 1], F32, "ncw")
    r.dparts = t2([P, NT, 5], F32, "dparts")
    r.dsum = t2([P, NT, 1], F32, "dsum")
    r.dtot = t2([P, NT], F32, "dtot")
    r.dinv = t2([P, NT], F32, "dinv")
    r.d = t2([P, NT], F32, "d")
    r.agc = t2([P, C], F32R, "agc")

    # DMAs issue immediately (SP/SWDGE queues; transfers overlap prev score)
    r.xraw = [sb.tile([P, HW], F32, tag="x", bufs=1, name="x") for _ in range(CT)]
    for ci in range(CT):
        nc.sync.dma_start(out=r.xraw[ci], in_=ftr[ci * P:(ci + 1) * P, :])
    nc.gpsimd.dma_start(out=r.convw, in_=convw[:, :])
    nc.sync.dma_start(out=r.convb, in_=convb[:, :])
    nc.gpsimd.dma_start(out=r.avgw, in_=avgw[:, :])
    nc.gpsimd.dma_start(out=r.maxw, in_=maxw[:, :])
    for t in range(CT):
        nc.gpsimd.dma_start(out=r.g[:, t, :], in_=gcnw[t * P:(t + 1) * P, :])
    nc.gpsimd.memset(r.dparts, 0.0)

    def slot():
        return apool.tile([P, BANK], F32, tag="s", name="sp")

    ops = []

    def mk_round(ci, half):
        def op():
            lo = half * (HW // 2)
            nc.gpsimd.tensor_copy(r.xr[:, ci, lo:lo + HW // 2],
                                  r.xraw[ci][:, lo:lo + HW // 2])
        return op
    for ci in range(CT):
        for half in range(2):
            ops.append(mk_round(ci, half))

    def op_round_g():
        nc.gpsimd.tensor_copy(r.gr[:, :, :], r.g[:, :, :])
    ops.append(op_round_g)

    def op_gplus():
        nc.vector.tensor_copy(r.gplus[:, :, :], r.g[:, :, :])
        for t in range(CT):
            blk = r.gplus[:, t, t * P:(t + 1) * P]
            nc.vector.tensor_add(blk, blk, identr)
    ops.append(op_gplus)

    def mk_wt(w_sb, wT):
        def op():
            ps = slot()
            for ci in range(CT):
                nc.tensor.transpose(ps[:, ci * P:(ci + 1) * P],
                                    w_sb[:, ci * P:(ci + 1) * P], ident)
            nc.vector.tensor_copy(wT[:, :, :], ps[:, :C])
        return op
    ops.append(mk_wt(r.convw, r.convwT))
    ops.append(mk_wt(r.avgw, r.avgwT))
    ops.append(mk_wt(r.maxw, r.maxwT))

    HH = HW // 2

    xf = r.xr.bitcast(F32)

    def mk_fold(ci):
        def op():
            nc.gpsimd.tensor_add(r.xfold[:, ci, :], xf[:, ci, 0:HH],
                                 xf[:, ci, HH:HW])
            nc.vector.tensor_max(r.mfold[:, ci, :], xf[:, ci, 0:HH],
                                  xf[:, ci, HH:HW])
        return op
    for ci in range(CT):
        ops.append(mk_fold(ci))

    def mk_red(ci):
        def op():
            nc.vector.reduce_sum(out=r.avg[:, ci, :], in_=r.xfold[:, ci, :],
                                 axis=AX.X)
            nc.vector.reduce_max(out=r.mx[:, ci, :], in_=r.mfold[:, ci, :],
                                 axis=AX.X)
        return op
    for ci in range(CT):
        ops.append(mk_red(ci))

    def mk_k(off, sz):
        def op():
            kps = slot()
            for ci in range(CT):
                nc.tensor.matmul(kps[:, :sz], lhsT=r.convwT[:, ci, :],
                                 rhs=r.xr[:, ci, off:off + sz],
                                 start=(ci == 0), stop=(ci == CT - 1))
            nc.vector.tensor_scalar(r.k[:, off:off + sz], kps[:, :sz],
                                    r.convb[:, :], 0.0, OP.add, OP.max)
        return op
    for off, sz in _chunks(HW, BANK):
        ops.append(mk_k(off, sz))

    def op_cw():
        aps = slot()
        for ci in range(CT):
            nc.tensor.matmul(aps[:, 0:1], lhsT=r.avgwT[:, ci, :],
                             rhs=r.avg[:, ci, :],
                             start=(ci == 0), stop=(ci == CT - 1))
        nc.scalar.activation(out=r.ra, in_=aps[:, 0:1], func=AF.Relu,
                             scale=1.0 / HW)
        mps = slot()
        for ci in range(CT):
            nc.tensor.matmul(mps[:, 0:1], lhsT=r.maxwT[:, ci, :],
                             rhs=r.mx[:, ci, :],
                             start=(ci == 0), stop=(ci == CT - 1))
        nc.scalar.activation(out=r.rm, in_=mps[:, 0:1], func=AF.Relu)
        nc.vector.tensor_add(r.cwin, r.ra, r.rm)
        nc.scalar.activation(out=r.cw, in_=r.cwin, func=AF.Sigmoid)
        nc.vector.tensor_scalar_mul(r.ncw, r.cw, -1.0)
    ops.append(op_cw)

    def mk_kq(off, sz):
        def op():
            nc.gpsimd.tensor_scalar_mul(r.kq[:, off:off + sz],
                                        r.k[:, off:off + sz], r.cw[:, :])
        return op
    for off, sz in _chunks(HW, 1152):
        ops.append(mk_kq(off, sz))

    def mk_kt(j0):
        nj = min(4, NT - j0)

        def op():
            tp = slot()
            for dj in range(nj):
                j = j0 + dj
                nc.tensor.transpose(tp[:, dj * P:(dj + 1) * P],
                                    r.k[:, j * P:(j + 1) * P].bitcast(F32),
                                    ident)
            nc.vector.tensor_copy(r.kT[:, j0:j0 + nj, :], tp[:, :nj * P])
        return op
    for j0 in range(0, NT, 4):
        ops.append(mk_kt(j0))

    def mk_ut(s0):
        ns = min(2, NT - s0)

        def op():
            up = slot()
            for ds in range(ns):
                sx = s0 + ds
                for ci in range(CT):
                    nc.tensor.matmul(up[:, ds * C:ds * C + C],
                                     lhsT=r.xr[:, ci, sx * P:(sx + 1) * P],
                                     rhs=r.gr[:, ci, :],
                                     start=(ci == 0), stop=(ci == CT - 1))
            nc.vector.tensor_copy(r.uT[:, s0:s0 + ns, :], up[:, :ns * C])
        return op
    for s0 in range(0, NT, 2):
        ops.append(mk_ut(s0))

    def mk_gx(s0):
        ns = min(2, NT - s0)

        def op():
            gp = slot()
            for ds in range(ns):
                sx = s0 + ds
                for ci in range(CT):
                    nc.tensor.matmul(gp[:, ds * C:ds * C + C],
                                     lhsT=r.xr[:, ci, sx * P:(sx + 1) * P],
                                     rhs=r.gplus[:, ci, :],
                                     start=(ci == 0), stop=(ci == CT - 1))
            nc.vector.tensor_copy(r.gxT[:, s0:s0 + ns, :], gp[:, :ns * C])
        return op
    for s0 in range(0, NT, 2):
        ops.append(mk_gx(s0))

    return r, ops


def _emit_score(tc, pools, consts, r, next_ops):
    """Upper-triangle sigmoid strips; interleaves next rep's head ops."""
    nc = tc.nc
    sb, scr, apool, gpool = pools
    ident, identr, q_sb, ones_sb = consts

    r.G_ps = gpool.tile([P, HW], F32, tag="G", name="G_ps")
    G_ps = r.G_ps
    g_last_strip = {b: min(NT - 2, 4 * b + 2)
                    for b in range((HW + BANK - 1) // BANK)}
    pending_g = []

    def emit_g():
        for i, abs_lo, sig, go, gl in pending_g:
            b = go // BANK
            first = (i == 0) and (go == max(P, b * BANK))
            is_bank_end = (go + gl == min(HW, (b + 1) * BANK))
            last = (i == g_last_strip[b]) and is_bank_end
            nc.tensor.matmul(
                G_ps[:, go:go + gl],
                lhsT=q_sb[:, P - 1 - i:2 * P - 1 - i],
                rhs=sig[:, go - abs_lo:go - abs_lo + gl],
                start=first, stop=last)
            if last:
                glo = max(P, b * BANK)
                ghi = min(HW, (b + 1) * BANK)
                nc.vector.tensor_copy(r.G[:, glo:ghi], G_ps[:, glo:ghi])
                # this bank's lower-triangle colsum totals: tiny matvecs
                # into the just-freed G_ps bank (no ring slot needed)
                js = [j for j in range(max(1, 4 * b), min(NT, 4 * (b + 1)))]
                for j in js:
                    idx = j - 4 * b
                    nc.tensor.matmul(G_ps[:, b * BANK + idx:b * BANK + idx + 1],
                                     lhsT=r.G[:, j * P:(j + 1) * P],
                                     rhs=ones_sb[:, :], start=True, stop=True)
        pending_g.clear()

    chunk_no = 0
    for i in range(NT):
        lhsT = r.kq[:, i * P:(i + 1) * P]
        row_lo = i * P
        for cidx, (coff, sz) in enumerate(_chunks(HW - row_lo, BANK)):
            abs_lo = row_lo + coff
            sp = apool.tile([P, BANK], F32, tag="s", name="sp")
            nc.tensor.matmul(sp[:, :sz], lhsT=lhsT,
                             rhs=r.k[:, abs_lo:abs_lo + sz],
                             start=True, stop=True)
            sig = scr.tile([P, BANK], BF16, tag="sig", name="sig")
            nc.scalar.activation(out=sig[:, :sz], in_=sp[:, :sz],
                                 func=AF.Sigmoid,
                                 accum_out=r.dparts[:, i, cidx:cidx + 1])
            lo = max(abs_lo, (i + 1) * P)
            hi = abs_lo + sz
            queued = []
            if lo < hi and i <= NT - 2:
                for go, gl in _bank_chunks(lo, hi):
                    queued.append((i, abs_lo, sig, go, gl))
            emit_g()
            pending_g.extend(queued)
            # interleave next rep's head (PE/DVE/Pool have slack vs ACT)
            chunk_no += 1
            if chunk_no >= 5 and next_ops:
                next_ops.pop(0)()
        emit_g()
    emit_g()


def _emit_tail(tc, pools, consts, r, drams):
    nc = tc.nc
    sb, scr, apool, gpool = pools
    ident, identr, q_sb, ones_sb = consts
    out = drams[6]

    tailt = gpool.tile([P, BANK], F32, tag="G", name="tailt")
    b_ps = tailt[:, 0:C]

    # d = (rowsum + colsum)^-1/2   (colsum totals precomputed mid-score,
    # scattered 4-per-bank in G_ps at stride BANK)
    gview = r.G_ps[:, 0:4 * BANK].rearrange("p (b i) -> p b i", i=BANK)
    nc.vector.tensor_copy(r.dcol[:, 1:4], r.G_ps[:, 1:4])
    nc.vector.tensor_copy(
        r.dcol.rearrange("p (b i) -> p b i", i=4)[:, 1:4, :], gview[:, 1:4, 0:4])
    nc.vector.tensor_copy(r.dcol[:, 16:18], r.G_ps[:, 4 * BANK:4 * BANK + 2])
    nc.vector.reduce_sum(out=r.dsum[:, :, :], in_=r.dparts[:, :, :], axis=AX.X)
    nc.vector.tensor_copy(r.dtot[:, 0:1], r.dsum[:, 0, :])
    nc.vector.tensor_add(r.dtot[:, 1:NT], r.dsum[:, 1:NT, 0], r.dcol[:, 1:NT])
    nc.vector.reciprocal(out=r.dinv[:, :], in_=r.dtot[:, :])
    nc.scalar.activation(out=r.d[:, :], in_=r.dinv[:, :], func=AF.Sqrt)

    # kdT = d * kT in place (gpsimd)
    for nt in range(NT):
        nc.gpsimd.tensor_scalar_mul(r.kT[:, nt, :], r.kT[:, nt, :],
                                    r.d[:, nt:nt + 1])

    # B = kdT^T uT;  agc = -cw * B
    for nt in range(NT):
        nc.tensor.matmul(b_ps, lhsT=r.kT[:, nt, :], rhs=r.uT[:, nt, :],
                         start=(nt == 0), stop=(nt == NT - 1))
    nc.vector.tensor_scalar_mul(r.agc[:, :], b_ps, r.ncw[:, :])

    # out^T strips = gxT + d * (k^T agc)   (gxT prefetched mid-score)
    for s in range(NT):
        sp = apool.tile([P, BANK], F32, tag="s", name="sp")
        z_ps = sp[:, 0:C]
        nc.tensor.matmul(z_ps, lhsT=r.k[:, s * P:(s + 1) * P],
                         rhs=r.agc[:, :], start=True, stop=True)
        nc.vector.scalar_tensor_tensor(r.outT[:, s, :], z_ps, r.d[:, s:s + 1],
                                       r.gxT[:, s, :], OP.mult, OP.add)
    # single fat output DMA; DRAM holds [p, s, c], host unpacks to [HW, C]
    nc.sync.dma_start(out=out[:, :], in_=r.outT[:, :, :])


_PROGRAM = None


def _get_program():
    global _PROGRAM
    if _PROGRAM is None:
        _PROGRAM = build_program()
    return _PROGRAM


def _in_maps(ftr, conv_k_w, conv_k_b, avg_fc_w, max_fc_w, gcn_w):
    wmaps = {
        "convw": np.ascontiguousarray(conv_k_w, dtype=np.float32),
        "convb": np.ascontiguousarray(np.asarray(conv_k_b, dtype=np.float32).reshape(M, 1)),
        "avgw": np.ascontiguousarray(avg_fc_w, dtype=np.float32),
        "maxw": np.ascontiguousarray(max_fc_w, dtype=np.float32),
        "gcnw": np.ascontiguousarray(gcn_w, dtype=np.float32),
    }
    return [
        {"ftr": np.ascontiguousarray(np.asarray(ftr[b], dtype=np.float32).reshape(C, HW)), **wmaps}
        for b in range(B)
    ]


def unpack_out(arr):
    """[P, NT*C] device layout -> [C, H, W] (row n = s*P + p)."""
    a = np.asarray(arr).reshape(P, NT, C).transpose(1, 0, 2).reshape(HW, C)
    return a.T.reshape(C, H, W)


def kernel(ftr, conv_k_w, conv_k_b, avg_fc_w, max_fc_w, gcn_w):
    nc = _get_program()
    in_maps = _in_maps(ftr, conv_k_w, conv_k_b, avg_fc_w, max_fc_w, gcn_w)
    res = run_bass_kernel_spmd(nc, in_maps, core_ids=list(range(N_CORES)))
    outs = [unpack_out(res.results[b]["out"]) for b in range(B)]
    return np.stack(outs, axis=0).astype(np.float32)

